# revision 8
# baseline (speedup 1.0000x reference)
"""Multi-head self-attention (N=2048, DIM=1024, NH=16, DK=64) on 8 trn2 cores.

Head-parallel sharding: core c computes heads 2c and 2c+1.
Per core: Q/K/V projections for its 128 head-dims, scores in [m, n] layout
(row-packed K=64 matmuls, both heads concurrent in the PE array), exp on ACT,
att = [V | 1]^T @ E accumulated over m-tiles (ones column yields the softmax
denominator as row 64), then transpose back to [n, d] via DMA-xbar and divide.
"""

import sys
from contextlib import ExitStack

import numpy as np

for _p in ("/opt/trn_rl_repo", "/root/.axon_site/_ro/trn_rl_repo"):
    if _p not in sys.path:
        sys.path.insert(0, _p)

import ml_dtypes  # noqa: E402

import concourse.bass as bass  # noqa: E402
import concourse.bacc as bacc  # noqa: E402
import concourse.mybir as mybir  # noqa: E402
import concourse.tile as tile  # noqa: E402
from concourse.bass import ds, ts  # noqa: E402
from concourse.bass_utils import run_bass_kernel_spmd  # noqa: E402

N = 2048
DIM = 1024
NH = 16
DK = 64
NCORES = 8
J = 128          # head dims per core (2 heads x 64)
KT = DIM // 128  # 8 contraction tiles
MT = N // 128    # 16 m-tiles
P = 128

F32 = mybir.dt.float32
BF16 = mybir.dt.bfloat16
EXP = mybir.ActivationFunctionType.Exp

_NC_CACHE = {}


def build_nc():
    nc = bacc.Bacc("TRN2", target_bir_lowering=False, debug=False)

    x_d = nc.dram_tensor("xt", [DIM, N], BF16, kind="ExternalInput")
    wq_d = nc.dram_tensor("wqt", [DIM, J], BF16, kind="ExternalInput")
    wk_d = nc.dram_tensor("wkt", [DIM, J], BF16, kind="ExternalInput")
    wv_d = nc.dram_tensor("wvt", [DIM, J], BF16, kind="ExternalInput")
    out_d = nc.dram_tensor("out", [N, J], F32, kind="ExternalOutput")

    with tile.TileContext(nc) as tc, ExitStack() as ctx:
        pers = ctx.enter_context(tc.tile_pool(name="pers", bufs=1))
        etp = ctx.enter_context(tc.tile_pool(name="et", bufs=4))
        vnp = ctx.enter_context(tc.tile_pool(name="vn", bufs=4))
        osbp = ctx.enter_context(tc.tile_pool(name="osb", bufs=4))
        ttp = ctx.enter_context(tc.tile_pool(name="tt", bufs=4))
        rcp = ctx.enter_context(tc.tile_pool(name="rc", bufs=8))
        outp = ctx.enter_context(tc.tile_pool(name="outp", bufs=4))
        stp = ctx.enter_context(
            tc.tile_pool(name="stp", bufs=2, space=bass.MemorySpace.PSUM)
        )
        opp = ctx.enter_context(
            tc.tile_pool(name="opp", bufs=4, space=bass.MemorySpace.PSUM)
        )

        # ---- persistent SBUF tensors
        x_sb = pers.tile([P, KT, N], BF16, tag="x")
        wq_sb = pers.tile([P, KT, J], BF16, tag="wq")
        wk_sb = pers.tile([P, KT, J], BF16, tag="wk")
        wv_sb = pers.tile([P, KT, J], BF16, tag="wv")
        qt_sb = pers.tile([P, N], BF16, tag="qt")
        kt_sb = pers.tile([P, N], BF16, tag="kt")
        vt_sb = pers.tile([P, N], BF16, tag="vt")
        vp_sb = pers.tile([P, MT, 2, DK + 1], BF16, tag="vp")
        wu_i = pers.tile([1, 1], F32, tag="wui")
        wu_o = pers.tile([1, 1], F32, tag="wuo")

        # ---- ACT exp-table warmup (overlaps the input DMA)
        nc.gpsimd.memset(wu_i[:, :], 0.0)
        nc.scalar.activation(wu_o[:, :], wu_i[:, :], EXP)

        # ones column for the attention matmul (denominator trick)
        nc.gpsimd.memset(vp_sb[:, :, :, :], 1.0)

        # ---- input DMAs: weights, then x^T by n-half so projections start early
        for w_sb, w_d in ((wq_sb, wq_d), (wk_sb, wk_d), (wv_sb, wv_d)):
            for k in range(KT):
                nc.sync.dma_start(w_sb[:, k, :], w_d[ts(k, P), :])
        for h in range(2):
            for k in range(KT):
                nc.sync.dma_start(
                    x_sb[:, k, ds(h * 1024, 1024)], x_d[ts(k, P), ds(h * 1024, 1024)]
                )

        def project(dst_sb, w_sb, n0):
            """dst_sb[:, n0:n0+512] = (w^T x^T) slice, accumulated over k."""
            ps = stp.tile([P, 512], F32, tag="st")
            for k in range(KT):
                nc.tensor.matmul(
                    ps[:, :],
                    w_sb[:, k, :],
                    x_sb[:, k, ds(n0, 512)],
                    start=(k == 0),
                    stop=(k == KT - 1),
                )
            nc.vector.tensor_copy(dst_sb[:, ds(n0, 512)], ps[:, :])

        def vprep(i):
            """Build V' tiles for m-tile i: transpose Vt block, split heads."""
            vn = vnp.tile([P, P], BF16, tag="vn")
            nc.sync.dma_start_transpose(vn[:, :], vt_sb[:, ts(i, P)])
            nc.vector.tensor_copy(vp_sb[:, i, 0, 0:DK], vn[:, 0:DK])
            nc.vector.tensor_copy(vp_sb[:, i, 1, 0:DK], vn[:, DK:2 * DK])

        def attn_iter(i, j, nb, o_ps):
            """One (m-tile, n-chunk) step: scores both heads, exp, att both heads."""
            n0 = nb * 1024 + j * 512
            st = stp.tile([P, 1024], F32, tag="st")
            # h0 in rows 0-63 of the PE array, h1 in rows 64-127 (concurrent)
            nc.tensor.matmul(
                st[:, 0:512],
                kt_sb[0:DK, ts(i, P)],
                qt_sb[0:DK, ds(n0, 512)],
                start=True, stop=True,
                tile_position=(0, 0),
            )
            nc.tensor.matmul(
                st[:, 512:1024],
                kt_sb[DK:2 * DK, ts(i, P)],
                qt_sb[DK:2 * DK, ds(n0, 512)],
                start=True, stop=True,
                tile_position=(64, 0),
            )
            et = etp.tile([P, 1024], BF16, tag="et")
            nc.scalar.activation(et[:, :], st[:, :], EXP)
            for h in range(2):
                nc.tensor.matmul(
                    o_ps[h][j][:, :],
                    vp_sb[:, i, h, :],
                    et[:, ds(h * 512, 512)],
                    start=(i == 0),
                    stop=(i == MT - 1),
                )

        def finalize(nb, o_ps):
            """Divide by row-sums, transpose to [n, d], DMA out."""
            for j in range(2):
                osb = [None, None]
                for h in range(2):
                    osb[h] = osbp.tile(
                        [P, 512], BF16, tag="osb", name=f"osb{nb}_{j}_{h}"
                    )
                    nc.gpsimd.memset(osb[h][DK:P, :], 0.0)
                    nc.vector.tensor_copy(osb[h][0:DK + 1, :], o_ps[h][j][:, :])
                for c in range(4):
                    ob = outp.tile([P, P], F32, tag="ob")
                    for h in range(2):
                        tt = ttp.tile([P, P], BF16, tag="tt")
                        nc.sync.dma_start_transpose(tt[:, :], osb[h][:, ts(c, P)])
                        rcf = rcp.tile([P, 1], F32, tag="rcf")
                        rcr = rcp.tile([P, 1], F32, tag="rcr")
                        nc.vector.tensor_copy(rcf[:, :], tt[:, DK:DK + 1])
                        nc.vector.reciprocal(rcr[:, :], rcf[:, :])
                        nc.vector.tensor_scalar_mul(
                            ob[:, ds(h * DK, DK)], tt[:, 0:DK], rcr[:, :]
                        )
                    nc.sync.dma_start(
                        out_d[ds(nb * 1024 + j * 512 + c * P, P), :], ob[:, :]
                    )

        # ---- projections (first n-half) + V' prep for m-tiles 0-7
        for n0 in (0, 512):
            project(qt_sb, wq_sb, n0)
        for n0 in (0, 512):
            project(kt_sb, wk_sb, n0)
        for n0 in (0, 512):
            project(vt_sb, wv_sb, n0)
        for i in range(8):
            vprep(i)

        # ---- block 0 (queries n 0..1023)
        o0 = [[opp.tile([DK + 1, 512], F32, tag="o", name=f"o0_{h}_{j}")
               for j in range(2)] for h in range(2)]  # [h][j]
        for i in range(MT):
            if i == 8:
                # second-half K/V projections, needed from m-tile 8 onwards
                for n0 in (1024, 1536):
                    project(kt_sb, wk_sb, n0)
                for n0 in (1024, 1536):
                    project(vt_sb, wv_sb, n0)
                for ii in range(8, MT):
                    vprep(ii)
            for j in range(2):
                attn_iter(i, j, 0, o0)
            if i == 11:
                project(qt_sb, wq_sb, 1024)
            if i == 13:
                project(qt_sb, wq_sb, 1536)
        finalize(0, o0)

        # ---- block 1 (queries n 1024..2047)
        o1 = [[opp.tile([DK + 1, 512], F32, tag="o", name=f"o1_{h}_{j}")
               for j in range(2)] for h in range(2)]
        for i in range(MT):
            for j in range(2):
                attn_iter(i, j, 1, o1)
        finalize(1, o1)

    nc.finalize()
    return nc


def kernel(x, rela, Wq, Wk, Wv):
    x = np.asarray(x, dtype=np.float32)
    Wq = np.asarray(Wq, dtype=np.float32)
    Wk = np.asarray(Wk, dtype=np.float32)
    Wv = np.asarray(Wv, dtype=np.float32)

    bf16 = ml_dtypes.bfloat16
    scale = 1.0 / np.sqrt(DK)
    xt = np.ascontiguousarray(x.T).astype(bf16)

    in_maps = []
    for c in range(NCORES):
        sl = slice(c * J, (c + 1) * J)
        in_maps.append({
            "xt": xt,
            "wqt": np.ascontiguousarray((Wq[sl, :] * scale).T).astype(bf16),
            "wkt": np.ascontiguousarray(Wk[sl, :].T).astype(bf16),
            "wvt": np.ascontiguousarray(Wv[sl, :].T).astype(bf16),
        })

    if "nc" not in _NC_CACHE:
        _NC_CACHE["nc"] = build_nc()
    res = run_bass_kernel_spmd(_NC_CACHE["nc"], in_maps, core_ids=list(range(NCORES)))
    out = np.concatenate([res.results[c]["out"] for c in range(NCORES)], axis=1)
    return np.ascontiguousarray(out.astype(np.float32))


if __name__ == "__main__":
    rng = np.random.default_rng(0)
    x = rng.standard_normal((N, DIM), dtype=np.float32)
    b = 1.0 / np.sqrt(DIM)
    Wq = rng.uniform(-b, b, (DIM, DIM)).astype(np.float32)
    Wk = rng.uniform(-b, b, (DIM, DIM)).astype(np.float32)
    Wv = rng.uniform(-b, b, (DIM, DIM)).astype(np.float32)
    out = kernel(x, np.zeros(1, np.float32), Wq, Wk, Wv)
    print(out.shape, out.dtype)


# revision 20
# speedup vs baseline: 1.1874x; 1.1874x over previous
"""Multi-head self-attention (N=2048, DIM=1024, NH=16, DK=64) on 8 trn2 cores.

Head-parallel sharding: core c computes heads 2c and 2c+1.
Per core: Q/K/V projections for its 128 head-dims, scores in [m, n] layout
(row-packed K=64 matmuls, both heads concurrent in the PE array), exp on ACT,
att = [V | 1]^T @ E accumulated over m-tiles (ones column yields the softmax
denominator as row 64), then transpose back to [n, d] via DMA-xbar and divide.
"""

import sys
from contextlib import ExitStack

import numpy as np

for _p in ("/opt/trn_rl_repo", "/root/.axon_site/_ro/trn_rl_repo"):
    if _p not in sys.path:
        sys.path.insert(0, _p)

import ml_dtypes  # noqa: E402

import concourse.bass as bass  # noqa: E402
import concourse.bacc as bacc  # noqa: E402
import concourse.mybir as mybir  # noqa: E402
import concourse.tile as tile  # noqa: E402
from concourse.bass import ds, ts  # noqa: E402
from concourse.bass_utils import run_bass_kernel_spmd  # noqa: E402
from concourse.masks import make_identity  # noqa: E402

N = 2048
DIM = 1024
NH = 16
DK = 64
NCORES = 8
J = 128          # head dims per core (2 heads x 64)
KT = DIM // 128  # 8 contraction tiles
MT = N // 128    # 16 m-tiles
P = 128

F32 = mybir.dt.float32
BF16 = mybir.dt.bfloat16
EXP = mybir.ActivationFunctionType.Exp

_NC_CACHE = {}


def build_nc():
    nc = bacc.Bacc("TRN2", target_bir_lowering=False, debug=False)

    x_d = nc.dram_tensor("xt", [DIM, N], BF16, kind="ExternalInput")
    # weights host-packed as [p, k*j]: element (p, k*J+j) = W^T[k*128+p, j]
    wq_d = nc.dram_tensor("wqt", [P, KT * J], BF16, kind="ExternalInput")
    wk_d = nc.dram_tensor("wkt", [P, KT * J], BF16, kind="ExternalInput")
    wv_d = nc.dram_tensor("wvt", [P, KT * J], BF16, kind="ExternalInput")
    out_d = nc.dram_tensor("out", [N, J], F32, kind="ExternalOutput")

    with tile.TileContext(nc) as tc, ExitStack() as ctx:
        pers = ctx.enter_context(tc.tile_pool(name="pers", bufs=1))
        etp = ctx.enter_context(tc.tile_pool(name="et", bufs=4))
        vnp = ctx.enter_context(tc.tile_pool(name="vn", bufs=4))
        osbp = ctx.enter_context(tc.tile_pool(name="osb", bufs=4))
        rcp = ctx.enter_context(tc.tile_pool(name="rc", bufs=8))
        outp = ctx.enter_context(tc.tile_pool(name="outp", bufs=4))
        stp = ctx.enter_context(
            tc.tile_pool(name="stp", bufs=2, space=bass.MemorySpace.PSUM)
        )
        opp = ctx.enter_context(
            tc.tile_pool(name="opp", bufs=4, space=bass.MemorySpace.PSUM)
        )

        # ---- persistent SBUF tensors
        x_sb = pers.tile([P, KT, N], BF16, tag="x")
        wq_sb = pers.tile([P, KT, J], BF16, tag="wq")
        wk_sb = pers.tile([P, KT, J], BF16, tag="wk")
        wv_sb = pers.tile([P, KT, J], BF16, tag="wv")
        qt_sb = pers.tile([P, N], BF16, tag="qt")
        kt_sb = pers.tile([P, N], BF16, tag="kt")
        vt_sb = pers.tile([P, N], BF16, tag="vt")
        vp_sb = pers.tile([P, MT, 2, DK + 1], BF16, tag="vp")
        ident = pers.tile([P, P], F32, tag="ident")
        wu_i = pers.tile([1, 1], F32, tag="wui")
        wu_o = pers.tile([1, 1], F32, tag="wuo")

        # ---- ACT exp-table warmup (overlaps the input DMA)
        nc.gpsimd.memset(wu_i[:, :], 0.0)
        nc.scalar.activation(wu_o[:, :], wu_i[:, :], EXP)

        # ones column for the attention matmul (denominator trick)
        nc.gpsimd.memset(vp_sb[:, :, :, :], 1.0)
        make_identity(nc, ident[:, :])

        # ---- input DMAs: weights (host-packed to match SBUF layout exactly),
        # then x^T by n-half so projections start early
        for w_sb, w_dd in ((wq_sb, wq_d), (wk_sb, wk_d), (wv_sb, wv_d)):
            nc.sync.dma_start(w_sb[:, :, :], w_dd[:, :])
        for h in range(2):
            for k in range(KT):
                nc.sync.dma_start(
                    x_sb[:, k, ds(h * 1024, 1024)], x_d[ts(k, P), ds(h * 1024, 1024)]
                )

        def project(dst_sb, w_sb, n0):
            """dst_sb[:, n0:n0+512] = (w^T x^T) slice, accumulated over k."""
            ps = stp.tile([P, 512], F32, tag="st")
            for k in range(KT):
                nc.tensor.matmul(
                    ps[:, :],
                    w_sb[:, k, :],
                    x_sb[:, k, ds(n0, 512)],
                    start=(k == 0),
                    stop=(k == KT - 1),
                )
            nc.vector.tensor_copy(dst_sb[:, ds(n0, 512)], ps[:, :])

        def vprep(i):
            """Build V' tiles for m-tile i: transpose Vt block, split heads."""
            vn = vnp.tile([P, P], BF16, tag="vn")
            nc.sync.dma_start_transpose(vn[:, :], vt_sb[:, ts(i, P)])
            nc.vector.tensor_copy(vp_sb[:, i, 0, 0:DK], vn[:, 0:DK])
            nc.vector.tensor_copy(vp_sb[:, i, 1, 0:DK], vn[:, DK:2 * DK])

        def attn_iter(i, j, nb, o_ps):
            """One (m-tile, n-chunk) step: scores both heads, exp, att both heads."""
            n0 = nb * 1024 + j * 512
            st = stp.tile([P, 1024], F32, tag="st")
            # h0 in rows 0-63 of the PE array, h1 in rows 64-127 (concurrent)
            nc.tensor.matmul(
                st[:, 0:512],
                kt_sb[0:DK, ts(i, P)],
                qt_sb[0:DK, ds(n0, 512)],
                start=True, stop=True,
                tile_position=(0, 0),
            )
            nc.tensor.matmul(
                st[:, 512:1024],
                kt_sb[DK:2 * DK, ts(i, P)],
                qt_sb[DK:2 * DK, ds(n0, 512)],
                start=True, stop=True,
                tile_position=(64, 0),
            )
            et = etp.tile([P, 1024], BF16, tag="et")
            nc.scalar.activation(et[:, :], st[:, :], EXP)
            for h in range(2):
                nc.tensor.matmul(
                    o_ps[h][j][:, :],
                    vp_sb[:, i, h, :],
                    et[:, ds(h * 512, 512)],
                    start=(i == 0),
                    stop=(i == MT - 1),
                )

        def finalize(nb, o_ps, tail):
            """Divide by row-sums, transpose to [n, d] via PE, DMA out.

            tail=True (last block): copies/muls on the now-idle ACT engine;
            otherwise on DVE so ACT keeps streaming exps of the next block.
            """
            for j in range(2):
                osb = [None, None]
                for h in range(2):
                    osb[h] = osbp.tile(
                        [DK + 1, 512], F32, tag="osb", name=f"osb{nb}_{j}_{h}"
                    )
                    if tail:
                        nc.scalar.copy(osb[h][:, :], o_ps[h][j][:, :])
                    else:
                        nc.vector.tensor_copy(osb[h][:, :], o_ps[h][j][:, :])
                for c in range(4):
                    ob = outp.tile([P, P], F32, tag="ob")
                    for h in range(2):
                        tt = stp.tile([P, DK + 1], F32, tag="st",
                                      name=f"tt{nb}_{j}_{c}_{h}")
                        nc.tensor.transpose(
                            tt[:, :], osb[h][:, ts(c, P)], ident[0:DK + 1, 0:DK + 1]
                        )
                        rcr = rcp.tile([P, 1], F32, tag="rcr")
                        nc.vector.reciprocal(rcr[:, :], tt[:, DK:DK + 1])
                        if tail:
                            nc.scalar.mul(ob[:, ds(h * DK, DK)], tt[:, 0:DK],
                                          rcr[:, :])
                        else:
                            nc.vector.tensor_scalar_mul(
                                ob[:, ds(h * DK, DK)], tt[:, 0:DK], rcr[:, :]
                            )
                    nc.sync.dma_start(
                        out_d[ds(nb * 1024 + j * 512 + c * P, P), :], ob[:, :]
                    )

        # ---- projections (first n-half) + V' prep for m-tiles 0-7
        for n0 in (0, 512):
            project(qt_sb, wq_sb, n0)
        for n0 in (0, 512):
            project(kt_sb, wk_sb, n0)
        for n0 in (0, 512):
            project(vt_sb, wv_sb, n0)
        for i in range(8):
            vprep(i)

        # ---- block 0 (queries n 0..1023)
        o0 = [[opp.tile([DK + 1, 512], F32, tag="o", name=f"o0_{h}_{j}")
               for j in range(2)] for h in range(2)]  # [h][j]
        for i in range(MT):
            if i == 8:
                # second-half K/V projections, needed from m-tile 8 onwards
                for n0 in (1024, 1536):
                    project(kt_sb, wk_sb, n0)
                for n0 in (1024, 1536):
                    project(vt_sb, wv_sb, n0)
                for ii in range(8, MT):
                    vprep(ii)
            for j in range(2):
                attn_iter(i, j, 0, o0)
            if i == 11:
                project(qt_sb, wq_sb, 1024)
            if i == 13:
                project(qt_sb, wq_sb, 1536)
        finalize(0, o0, tail=False)

        # ---- block 1 (queries n 1024..2047)
        o1 = [[opp.tile([DK + 1, 512], F32, tag="o", name=f"o1_{h}_{j}")
               for j in range(2)] for h in range(2)]
        for i in range(MT):
            for j in range(2):
                attn_iter(i, j, 1, o1)
        finalize(1, o1, tail=True)

    nc.finalize()
    return nc


def make_in_maps(x, Wq, Wk, Wv):
    x = np.asarray(x, dtype=np.float32)
    Wq = np.asarray(Wq, dtype=np.float32)
    Wk = np.asarray(Wk, dtype=np.float32)
    Wv = np.asarray(Wv, dtype=np.float32)

    bf16 = ml_dtypes.bfloat16
    scale = 1.0 / np.sqrt(DK)
    xt = np.ascontiguousarray(x.T).astype(bf16)

    def pack_w(w_slice):
        # [DIM, J] -> [P, KT*J]: element (p, k*J+j) = W^T[k*P+p, j]
        wt = w_slice.T.reshape(KT, P, J).transpose(1, 0, 2).reshape(P, KT * J)
        return np.ascontiguousarray(wt).astype(bf16)

    in_maps = []
    for c in range(NCORES):
        sl = slice(c * J, (c + 1) * J)
        in_maps.append({
            "xt": xt,
            "wqt": pack_w(Wq[sl, :] * scale),
            "wkt": pack_w(Wk[sl, :]),
            "wvt": pack_w(Wv[sl, :]),
        })
    return in_maps


def kernel(x, rela, Wq, Wk, Wv):
    in_maps = make_in_maps(x, Wq, Wk, Wv)
    if "nc" not in _NC_CACHE:
        _NC_CACHE["nc"] = build_nc()
    res = run_bass_kernel_spmd(_NC_CACHE["nc"], in_maps, core_ids=list(range(NCORES)))
    out = np.concatenate([res.results[c]["out"] for c in range(NCORES)], axis=1)
    return np.ascontiguousarray(out.astype(np.float32))


if __name__ == "__main__":
    rng = np.random.default_rng(0)
    x = rng.standard_normal((N, DIM), dtype=np.float32)
    b = 1.0 / np.sqrt(DIM)
    Wq = rng.uniform(-b, b, (DIM, DIM)).astype(np.float32)
    Wk = rng.uniform(-b, b, (DIM, DIM)).astype(np.float32)
    Wv = rng.uniform(-b, b, (DIM, DIM)).astype(np.float32)
    out = kernel(x, np.zeros(1, np.float32), Wq, Wk, Wv)
    print(out.shape, out.dtype)
